# revision 7
# baseline (speedup 1.0000x reference)
"""CTC loss (log_softmax + CTC forward DP, torch 'mean' reduction) on 8 Trainium2 cores.

Strategy — data-parallel over batch (B=64 -> 8 batches per core):

Device, per core:
  * Streams its pred shard ([2048, 6625] f32, ~54 MB) through SBUF once.
    ScalarE computes exp(x) with a fused per-row accumulate, producing
    Z[row] = sum_c exp(pred[row, c])  (log-softmax denominator; logits are
    ~N(0,1) so the max-subtraction is unnecessary for fp32 exp).
  * Runs the CTC forward DP in the scaled linear domain on VectorE,
    concurrently with the DMA/ScalarE stream (the DP only touches the tiny
    host-gathered q tensors, so the two pipelines are independent).

    Fast path (no repeated adjacent labels): states are stored parity-packed
    [pad | odd(25) | even(26)], which folds the CTC skip-transition mask into
    a shared subexpression -> 3 VectorE tensor_tensor ops per time step:
        P[j] = E[j] + O[j-1]               (even-state bracket, also feeds odd)
        t[j] = O[j] + P[j]                 (odd-state bracket incl. skip)
        A' = [t | P] * q_packed[t]         (one fused multiply)
    Every 8 steps alpha is renormalized by its row max; the 1/max scale is
    applied by ScalarE to a *future* q slice so the VectorE chain never
    stalls on it.

    Fallback (repeats present, rare): plain 4-op/step update in state order
    with a separately masked qm = q * skip_ok.

Host (cheap, index-dependent prep + final scalar combine):
  * Extended labels, the 51-column gather per (b, t) (indices depend only on
    targets), validity/skip masks folded in as exp(-1000) = 0, exp() of the
    tiny gathered tensor, parity packing.
  * Final per-batch loss:  -log(A_T[2l] + A_T[2l-1]) - sum(log renorms)
    + sum_t log Z[b, t], divided by target length, averaged over batches.
"""

import os
import sys

for _p in ("/opt/trn_rl_repo", "/root/.axon_site/_ro/trn_rl_repo"):
    if os.path.isdir(_p) and _p not in sys.path:
        sys.path.insert(0, _p)
        break

import numpy as np

import concourse.bacc as bacc
import concourse.mybir as mybir
import concourse.tile as tile
from concourse import bass_utils

F32 = mybir.dt.float32

# Problem constants (hardcoded per the harness contract).
B = 64
T = 256
C = 6625
L = 25
S = 2 * L + 1  # 51 extended-label states
NCORES = 8
BSH = B // NCORES  # 8 batches per core
RENORM = 8  # renormalize alpha every RENORM time steps
NEG = -1000.0  # additive mask; exp(-1000) == 0 in fp32

X = mybir.AxisListType.X
MAX = mybir.AluOpType.max
EXP = mybir.ActivationFunctionType.Exp


def _new_nc():
    # Bacc (not raw Bass): its compile() pass legalizes multi-semaphore
    # waits via event semaphores — walrus rejects >1 sync wait per
    # instruction otherwise.
    return bacc.Bacc(
        "TRN2",
        target_bir_lowering=False,
        debug=False,
        enable_asserts=False,
        num_devices=NCORES,
    )


def _stream_softmax_denominator(nc, tc, sp, pred_d, zbuf, bsh, t, c):
    """DMA the pred shard tile-by-tile; ScalarE exp with per-row accumulate."""
    rows = bsh * t
    nt = rows // 128
    predv = pred_d.ap().rearrange("(n p) c -> n p c", p=128)
    for i in range(nt):
        ptile = sp.tile([128, c], F32, name="ptile", tag="ptile")
        # alternate HWDGE (sync) and SWDGE (gpsimd) queues so transfers
        # on different rings overlap queue turnaround
        eng = nc.sync if i % 2 == 0 else nc.gpsimd
        eng.dma_start(out=ptile, in_=predv[i])
        nc.scalar.activation(ptile, ptile, EXP, accum_out=zbuf[:, i : i + 1])


def build_fast(bsh=BSH, t=T, c=C, l=L, renorm=RENORM):
    """Parity-packed 3-op/step DP. Valid only when no batch has repeated
    adjacent labels inside its target length (host checks and falls back)."""
    s = 2 * l + 1
    n_o, n_e = l, l + 1  # odd / even state counts
    rows = bsh * t
    assert rows % 128 == 0
    nt = rows // 128
    # renorm events at t % renorm == renorm-1, t <= t-1-2 (scale lands on q[t+2])
    renorm_ts = [tt for tt in range(1, t) if tt % renorm == renorm - 1 and tt + 2 < t]
    nre = len(renorm_ts)

    nc = _new_nc()
    pred_d = nc.dram_tensor("pred", [rows, c], F32, kind="ExternalInput")
    qp_d = nc.dram_tensor("qp", [bsh, t * s], F32, kind="ExternalInput")
    z_d = nc.dram_tensor("zsums", [128, nt], F32, kind="ExternalOutput")
    a_d = nc.dram_tensor("alphaT", [bsh, s + 1], F32, kind="ExternalOutput")
    r_d = nc.dram_tensor("rmaxs", [bsh, nre], F32, kind="ExternalOutput")

    with tile.TileContext(nc) as tc:
        with (
            tc.tile_pool(name="persist", bufs=1) as pp,
            tc.tile_pool(name="stream", bufs=2) as sp,
            tc.tile_pool(name="dp", bufs=4) as dpp,
        ):
            qp = pp.tile([bsh, t * s], F32, name="qp")
            zbuf = pp.tile([128, nt], F32, name="zbuf")
            rbuf = pp.tile([bsh, nre], F32, name="rbuf")
            a0 = pp.tile([bsh, s + 1], F32, name="a0")
            a1 = pp.tile([bsh, s + 1], F32, name="a1")

            nc.sync.dma_start(out=qp, in_=qp_d.ap())

            # alpha layout: col 0 pad, cols 1..n_o = odd states A[1],A[3],..,
            # cols 1+n_o..s = even states A[0],A[2],..
            # init: A[1] = q0[state1] (packed idx 0), A[0] = q0[state0] (idx n_o)
            nc.vector.memset(a0, 0.0)
            nc.vector.memset(a1, 0.0)
            nc.scalar.copy(a0[:, 1:2], qp[:, 0:1])
            nc.scalar.copy(a0[:, 1 + n_o : 2 + n_o], qp[:, n_o : n_o + 1])

            _stream_softmax_denominator(nc, tc, sp, pred_d, zbuf, bsh, t, c)

            cur, nxt = a0, a1
            staged = {}  # step -> pre-scaled q slice tile
            jr = 0
            for tt in range(1, t):
                scr = dpp.tile([bsh, s], F32, name="scr", tag="scr")
                # P[j] = E[j] + O[j-1]  (j = 0..n_e-1)
                nc.vector.tensor_add(
                    scr[:, n_o : n_o + n_e], cur[:, 1 + n_o : 1 + n_o + n_e],
                    cur[:, 0:n_e],
                )
                # t[j] = O[j] + P[j]  (j = 0..n_o-1)
                nc.vector.tensor_add(
                    scr[:, 0:n_o], cur[:, 1 : 1 + n_o], scr[:, n_o : n_o + n_o]
                )
                qsrc = staged.pop(tt, None)
                if qsrc is None:
                    qsrc = qp[:, tt * s : (tt + 1) * s]
                nc.vector.tensor_mul(nxt[:, 1 : 1 + s], scr, qsrc)
                if renorm_ts and tt == renorm_ts[min(jr, nre - 1)] and jr < nre:
                    rm = rbuf[:, jr : jr + 1]
                    nc.vector.tensor_reduce(rm, nxt[:, 1 : 1 + s], X, MAX)
                    rcp = dpp.tile([bsh, 1], F32, name="rcp", tag="rcp")
                    nc.vector.reciprocal(rcp, rm)
                    # apply the scale on ScalarE to the q slice two steps
                    # ahead — off the VectorE critical chain
                    stg = dpp.tile([bsh, s], F32, name="stg", tag="stg")
                    nc.scalar.mul(stg, qp[:, (tt + 2) * s : (tt + 3) * s], rcp)
                    staged[tt + 2] = stg
                    jr += 1
                cur, nxt = nxt, cur
            assert jr == nre and not staged

            nc.sync.dma_start(out=a_d.ap(), in_=cur)
            nc.sync.dma_start(out=r_d.ap(), in_=rbuf)
            nc.sync.dma_start(out=z_d.ap(), in_=zbuf)
    nc.compile()
    return nc


def build_fallback(bsh=BSH, t=T, c=C, l=L, renorm=RENORM):
    """State-order 4-op/step DP with explicit skip-masked qm. Handles
    repeated adjacent labels exactly."""
    s = 2 * l + 1
    rows = bsh * t
    assert rows % 128 == 0
    nt = rows // 128
    nre = t // renorm

    nc = _new_nc()
    pred_d = nc.dram_tensor("pred", [rows, c], F32, kind="ExternalInput")
    q_d = nc.dram_tensor("q", [bsh, t * s], F32, kind="ExternalInput")
    qm_d = nc.dram_tensor("qm", [bsh, t * s], F32, kind="ExternalInput")
    z_d = nc.dram_tensor("zsums", [128, nt], F32, kind="ExternalOutput")
    a_d = nc.dram_tensor("alphaT", [bsh, s + 2], F32, kind="ExternalOutput")
    r_d = nc.dram_tensor("rmaxs", [bsh, nre], F32, kind="ExternalOutput")

    with tile.TileContext(nc) as tc:
        with (
            tc.tile_pool(name="persist", bufs=1) as pp,
            tc.tile_pool(name="stream", bufs=2) as sp,
            tc.tile_pool(name="dp", bufs=4) as dpp,
        ):
            q = pp.tile([bsh, t * s], F32, name="q")
            qm = pp.tile([bsh, t * s], F32, name="qm")
            zbuf = pp.tile([128, nt], F32, name="zbuf")
            rbuf = pp.tile([bsh, nre], F32, name="rbuf")
            a0 = pp.tile([bsh, s + 2], F32, name="a0")
            a1 = pp.tile([bsh, s + 2], F32, name="a1")

            nc.sync.dma_start(out=q, in_=q_d.ap())
            nc.sync.dma_start(out=qm, in_=qm_d.ap())

            nc.vector.memset(a0, 0.0)
            nc.vector.memset(a1, 0.0)
            nc.scalar.copy(a0[:, 2:4], q[:, 0:2])

            _stream_softmax_denominator(nc, tc, sp, pred_d, zbuf, bsh, t, c)

            cur, nxt = a0, a1
            jr = 0
            for tt in range(1, t):
                qt = q[:, tt * s : (tt + 1) * s]
                mqt = qm[:, tt * s : (tt + 1) * s]
                u = dpp.tile([bsh, s], F32, name="u", tag="u")
                uq = dpp.tile([bsh, s], F32, name="uq", tag="uq")
                w = dpp.tile([bsh, s], F32, name="w", tag="w")
                nc.vector.tensor_add(u, cur[:, 2 : 2 + s], cur[:, 1 : 1 + s])
                nc.vector.tensor_mul(uq, u, qt)
                nc.vector.tensor_mul(w, cur[:, 0:s], mqt)
                nc.vector.tensor_add(nxt[:, 2 : 2 + s], uq, w)
                if tt % renorm == renorm - 1:
                    rm = rbuf[:, jr : jr + 1]
                    nc.vector.tensor_reduce(rm, nxt[:, 2 : 2 + s], X, MAX)
                    rcp = dpp.tile([bsh, 1], F32, name="rcp", tag="rcp")
                    nc.vector.reciprocal(rcp, rm)
                    nc.vector.tensor_scalar_mul(
                        nxt[:, 2 : 2 + s], nxt[:, 2 : 2 + s], rcp
                    )
                    jr += 1
                cur, nxt = nxt, cur
            assert jr == nre

            nc.sync.dma_start(out=a_d.ap(), in_=cur)
            nc.sync.dma_start(out=r_d.ap(), in_=rbuf)
            nc.sync.dma_start(out=z_d.ap(), in_=zbuf)
    nc.compile()
    return nc


def host_prepare(pred, targets, target_lengths, bsh=BSH, t=T, l=L):
    """Index-dependent prep. Returns (mode, per-core input maps)."""
    s = 2 * l + 1
    b = pred.shape[0]
    ncores = b // bsh
    targets = np.asarray(targets)
    lengths = np.asarray(target_lengths)

    ext = np.zeros((b, s), dtype=np.int64)
    ext[:, 1::2] = targets
    ext_m2 = np.pad(ext[:, :-2], ((0, 0), (2, 0)))
    skip_ok = (np.arange(s)[None, :] >= 2) & (ext != 0) & (ext != ext_m2)
    # states beyond 2*len are invalid; zeroing them in q keeps them exactly 0
    # in the DP so the periodic renorm max is over valid states only
    valid = np.arange(s)[None, :] <= 2 * lengths[:, None]

    raw = np.take_along_axis(pred, ext[:, None, :], axis=2)  # [B, T, S]
    q = np.where(valid[:, None, :], np.exp(raw, dtype=np.float32), 0.0).astype(
        np.float32
    )

    # repeats only matter inside the target length
    rep = targets[:, 1:] == targets[:, :-1]
    inlen = (np.arange(1, l)[None, :] < lengths[:, None])
    has_repeats = bool(np.any(rep & inlen))

    in_maps = []
    if not has_repeats:
        order = np.concatenate([np.arange(1, s, 2), np.arange(0, s, 2)])  # odd|even
        qp = q[:, :, order]
        for k in range(ncores):
            sl = slice(k * bsh, (k + 1) * bsh)
            in_maps.append(
                {
                    "pred": np.ascontiguousarray(pred[sl].reshape(bsh * t, -1)),
                    "qp": np.ascontiguousarray(qp[sl].reshape(bsh, t * s)),
                }
            )
        return "fast", in_maps

    qm = np.where(skip_ok[:, None, :], q, 0.0).astype(np.float32)
    for k in range(ncores):
        sl = slice(k * bsh, (k + 1) * bsh)
        in_maps.append(
            {
                "pred": np.ascontiguousarray(pred[sl].reshape(bsh * t, -1)),
                "q": np.ascontiguousarray(q[sl].reshape(bsh, t * s)),
                "qm": np.ascontiguousarray(qm[sl].reshape(bsh, t * s)),
            }
        )
    return "fallback", in_maps


def host_finish(mode, results, target_lengths, bsh=BSH, t=T, l=L):
    """Combine per-core device outputs into the scalar mean CTC loss."""
    b = len(results) * bsh
    acc = 0.0
    for k, res in enumerate(results):
        a = res["alphaT"].astype(np.float64)
        z = res["zsums"].astype(np.float64)
        r = res["rmaxs"].astype(np.float64)
        logz = np.log(z.T.reshape(-1))  # row-major per-core log Z
        for j in range(bsh):
            bl = int(target_lengths[k * bsh + j])
            lse_sum = logz[j * t : (j + 1) * t].sum()
            logscale = np.log(r[j]).sum()
            if mode == "fast":
                # parity layout: A[2l] = even part col 1+l+bl ; A[2l-1] = col bl
                val = a[j, 1 + l + bl] + a[j, bl]
            else:
                val = a[j, 2 + 2 * bl] + a[j, 2 + 2 * bl - 1]
            with np.errstate(divide="ignore"):
                loss_b = -(np.log(val) + logscale - lse_sum)
            if not np.isfinite(loss_b) or loss_b > 1e29:
                loss_b = 0.0  # zero_infinity
            acc += loss_b / max(bl, 1)
    return np.float32(acc / b)


_NC_CACHE = {}


def _get_nc(mode):
    if mode not in _NC_CACHE:
        _NC_CACHE[mode] = build_fast() if mode == "fast" else build_fallback()
    return _NC_CACHE[mode]


def run_device(mode, in_maps, trace=False, **kwargs):
    nc = _get_nc(mode)
    return bass_utils.run_bass_kernel_spmd(
        nc, in_maps, core_ids=list(range(NCORES)), trace=trace, **kwargs
    )


def kernel(pred, targets, target_lengths):
    pred = np.asarray(pred, dtype=np.float32)
    mode, in_maps = host_prepare(pred, targets, target_lengths)
    res = run_device(mode, in_maps)
    return host_finish(mode, res.results, np.asarray(target_lengths))


# revision 16
# speedup vs baseline: 1.1977x; 1.1977x over previous
"""CTC loss (log_softmax + CTC forward DP, torch 'mean' reduction) on 8 Trainium2 cores.

Strategy — data-parallel over batch (B=64 -> 8 batches per core):

Device, per core:
  * Streams its pred shard ([2048, 6625] f32, ~54 MB) through SBUF once.
    ScalarE computes exp(x) with a fused per-row accumulate, producing
    Z[row] = sum_c exp(pred[row, c])  (log-softmax denominator; logits are
    ~N(0,1) so the max-subtraction is unnecessary for fp32 exp).
  * Runs the CTC forward DP in the scaled linear domain on VectorE,
    concurrently with the DMA/ScalarE stream (the DP only touches the tiny
    host-gathered q tensors, so the two pipelines are independent).

    Fast path (no repeated adjacent labels): states are stored parity-packed
    [pad | odd(25) | even(26)], which folds the CTC skip-transition mask into
    a shared subexpression -> 3 VectorE tensor_tensor ops per time step:
        P[j] = E[j] + O[j-1]               (even-state bracket, also feeds odd)
        t[j] = O[j] + P[j]                 (odd-state bracket incl. skip)
        A' = [t | P] * q_packed[t]         (one fused multiply)
    Every 8 steps alpha is renormalized by its row max; the 1/max scale is
    applied by ScalarE to a *future* q slice so the VectorE chain never
    stalls on it.

    Fallback (repeats present, rare): plain 4-op/step update in state order
    with a separately masked qm = q * skip_ok.

Host (cheap, index-dependent prep + final scalar combine):
  * Extended labels, the 51-column gather per (b, t) (indices depend only on
    targets), validity/skip masks folded in as exp(-1000) = 0, exp() of the
    tiny gathered tensor, parity packing.
  * Final per-batch loss:  -log(A_T[2l] + A_T[2l-1]) - sum(log renorms)
    + sum_t log Z[b, t], divided by target length, averaged over batches.
"""

import os
import sys

for _p in ("/opt/trn_rl_repo", "/root/.axon_site/_ro/trn_rl_repo"):
    if os.path.isdir(_p) and _p not in sys.path:
        sys.path.insert(0, _p)
        break

import numpy as np

import concourse.bacc as bacc
import concourse.mybir as mybir
import concourse.tile as tile
from concourse import bass_utils

F32 = mybir.dt.float32

# Problem constants (hardcoded per the harness contract).
B = 64
T = 256
C = 6625
L = 25
S = 2 * L + 1  # 51 extended-label states
NCORES = 8
BSH = B // NCORES  # 8 batches per core
RENORM = 16  # renormalize alpha every RENORM time steps
NEG = -1000.0  # additive mask; exp(-1000) == 0 in fp32

X = mybir.AxisListType.X
MAX = mybir.AluOpType.max
EXP = mybir.ActivationFunctionType.Exp


def _new_nc():
    # Bacc (not raw Bass): its compile() pass legalizes multi-semaphore
    # waits via event semaphores — walrus rejects >1 sync wait per
    # instruction otherwise.
    return bacc.Bacc(
        "TRN2",
        target_bir_lowering=False,
        debug=False,
        enable_asserts=False,
        num_devices=NCORES,
    )


def _stream_softmax_denominator(nc, tc, sp, pred_d, zbuf, bsh, t, c):
    """DMA the pred shard tile-by-tile; ScalarE exp with per-row accumulate."""
    rows = bsh * t
    nt = rows // 128
    predv = pred_d.ap().rearrange("(n p) c -> n p c", p=128)
    for i in range(nt):
        ptile = sp.tile([128, c], F32, name="ptile", tag="ptile")
        # alternate HWDGE (sync) and SWDGE (gpsimd) queues so transfers
        # on different rings overlap queue turnaround
        eng = nc.sync if i % 2 == 0 else nc.gpsimd
        eng.dma_start(out=ptile, in_=predv[i])
        nc.scalar.activation(ptile, ptile, EXP, accum_out=zbuf[:, i : i + 1])


def build_fast(bsh=BSH, t=T, c=C, l=L, renorm=RENORM):
    """Parity-packed 3-op/step DP. Valid only when no batch has repeated
    adjacent labels inside its target length (host checks and falls back)."""
    s = 2 * l + 1
    n_o, n_e = l, l + 1  # odd / even state counts
    rows = bsh * t
    assert rows % 128 == 0
    nt = rows // 128
    nre = t // renorm

    nc = _new_nc()
    pred_d = nc.dram_tensor("pred", [rows, c], F32, kind="ExternalInput")
    qp_d = nc.dram_tensor("qp", [bsh, t * s], F32, kind="ExternalInput")
    z_d = nc.dram_tensor("zsums", [128, nt], F32, kind="ExternalOutput")
    a_d = nc.dram_tensor("alphaT", [bsh, s + 1], F32, kind="ExternalOutput")
    r_d = nc.dram_tensor("rmaxs", [bsh, nre], F32, kind="ExternalOutput")

    with tile.TileContext(nc) as tc:
        with (
            tc.tile_pool(name="persist", bufs=1) as pp,
            tc.tile_pool(name="stream", bufs=2) as sp,
            tc.tile_pool(name="dp", bufs=4) as dpp,
        ):
            qp = pp.tile([bsh, t * s], F32, name="qp")
            zbuf = pp.tile([128, nt], F32, name="zbuf")
            rbuf = pp.tile([bsh, nre], F32, name="rbuf")
            a0 = pp.tile([bsh, s + 1], F32, name="a0")
            a1 = pp.tile([bsh, s + 1], F32, name="a1")

            nc.sync.dma_start(out=qp, in_=qp_d.ap())

            # alpha layout: col 0 pad, cols 1..n_o = odd states A[1],A[3],..,
            # cols 1+n_o..s = even states A[0],A[2],..
            # init: A[1] = q0[state1] (packed idx 0), A[0] = q0[state0] (idx n_o)
            nc.vector.memset(a0, 0.0)
            nc.vector.memset(a1, 0.0)
            nc.scalar.copy(a0[:, 1:2], qp[:, 0:1])
            nc.scalar.copy(a0[:, 1 + n_o : 2 + n_o], qp[:, n_o : n_o + 1])

            _stream_softmax_denominator(nc, tc, sp, pred_d, zbuf, bsh, t, c)

            cur, nxt = a0, a1
            jr = 0
            for tt in range(1, t):
                scr = dpp.tile([bsh, s], F32, name="scr", tag="scr")
                # P[j] = E[j] + O[j-1]  (j = 0..n_e-1)
                nc.vector.tensor_add(
                    scr[:, n_o : n_o + n_e], cur[:, 1 + n_o : 1 + n_o + n_e],
                    cur[:, 0:n_e],
                )
                # t[j] = O[j] + P[j]  (j = 0..n_o-1)
                nc.vector.tensor_add(
                    scr[:, 0:n_o], cur[:, 1 : 1 + n_o], scr[:, n_o : n_o + n_o]
                )
                nc.vector.tensor_mul(
                    nxt[:, 1 : 1 + s], scr, qp[:, tt * s : (tt + 1) * s]
                )
                if tt % renorm == renorm - 1:
                    rm = rbuf[:, jr : jr + 1]
                    nc.vector.tensor_reduce(rm, nxt[:, 1 : 1 + s], X, MAX)
                    rcp = dpp.tile([bsh, 1], F32, name="rcp", tag="rcp")
                    nc.vector.reciprocal(rcp, rm)
                    nc.vector.tensor_scalar_mul(
                        nxt[:, 1 : 1 + s], nxt[:, 1 : 1 + s], rcp
                    )
                    jr += 1
                cur, nxt = nxt, cur
            assert jr == nre

            nc.sync.dma_start(out=a_d.ap(), in_=cur)
            nc.sync.dma_start(out=r_d.ap(), in_=rbuf)
            nc.sync.dma_start(out=z_d.ap(), in_=zbuf)
    nc.compile()
    return nc


def build_fallback(bsh=BSH, t=T, c=C, l=L, renorm=RENORM):
    """State-order 4-op/step DP with explicit skip-masked qm. Handles
    repeated adjacent labels exactly."""
    s = 2 * l + 1
    rows = bsh * t
    assert rows % 128 == 0
    nt = rows // 128
    nre = t // renorm

    nc = _new_nc()
    pred_d = nc.dram_tensor("pred", [rows, c], F32, kind="ExternalInput")
    q_d = nc.dram_tensor("q", [bsh, t * s], F32, kind="ExternalInput")
    qm_d = nc.dram_tensor("qm", [bsh, t * s], F32, kind="ExternalInput")
    z_d = nc.dram_tensor("zsums", [128, nt], F32, kind="ExternalOutput")
    a_d = nc.dram_tensor("alphaT", [bsh, s + 2], F32, kind="ExternalOutput")
    r_d = nc.dram_tensor("rmaxs", [bsh, nre], F32, kind="ExternalOutput")

    with tile.TileContext(nc) as tc:
        with (
            tc.tile_pool(name="persist", bufs=1) as pp,
            tc.tile_pool(name="stream", bufs=2) as sp,
            tc.tile_pool(name="dp", bufs=4) as dpp,
        ):
            q = pp.tile([bsh, t * s], F32, name="q")
            qm = pp.tile([bsh, t * s], F32, name="qm")
            zbuf = pp.tile([128, nt], F32, name="zbuf")
            rbuf = pp.tile([bsh, nre], F32, name="rbuf")
            a0 = pp.tile([bsh, s + 2], F32, name="a0")
            a1 = pp.tile([bsh, s + 2], F32, name="a1")

            nc.sync.dma_start(out=q, in_=q_d.ap())
            nc.sync.dma_start(out=qm, in_=qm_d.ap())

            nc.vector.memset(a0, 0.0)
            nc.vector.memset(a1, 0.0)
            nc.scalar.copy(a0[:, 2:4], q[:, 0:2])

            _stream_softmax_denominator(nc, tc, sp, pred_d, zbuf, bsh, t, c)

            cur, nxt = a0, a1
            jr = 0
            for tt in range(1, t):
                qt = q[:, tt * s : (tt + 1) * s]
                mqt = qm[:, tt * s : (tt + 1) * s]
                u = dpp.tile([bsh, s], F32, name="u", tag="u")
                uq = dpp.tile([bsh, s], F32, name="uq", tag="uq")
                w = dpp.tile([bsh, s], F32, name="w", tag="w")
                nc.vector.tensor_add(u, cur[:, 2 : 2 + s], cur[:, 1 : 1 + s])
                nc.vector.tensor_mul(uq, u, qt)
                nc.vector.tensor_mul(w, cur[:, 0:s], mqt)
                nc.vector.tensor_add(nxt[:, 2 : 2 + s], uq, w)
                if tt % renorm == renorm - 1:
                    rm = rbuf[:, jr : jr + 1]
                    nc.vector.tensor_reduce(rm, nxt[:, 2 : 2 + s], X, MAX)
                    rcp = dpp.tile([bsh, 1], F32, name="rcp", tag="rcp")
                    nc.vector.reciprocal(rcp, rm)
                    nc.vector.tensor_scalar_mul(
                        nxt[:, 2 : 2 + s], nxt[:, 2 : 2 + s], rcp
                    )
                    jr += 1
                cur, nxt = nxt, cur
            assert jr == nre

            nc.sync.dma_start(out=a_d.ap(), in_=cur)
            nc.sync.dma_start(out=r_d.ap(), in_=rbuf)
            nc.sync.dma_start(out=z_d.ap(), in_=zbuf)
    nc.compile()
    return nc


def host_prepare(pred, targets, target_lengths, bsh=BSH, t=T, l=L):
    """Index-dependent prep. Returns (mode, per-core input maps, csum) where
    csum[b] = sum_t log(max_s q[b,t,s]) — the per-step normalizer folded out
    of q so the on-device alpha growth is deterministically <= 3 per step
    (renorm then only needs to run every RENORM=16 steps)."""
    s = 2 * l + 1
    b = pred.shape[0]
    ncores = b // bsh
    targets = np.asarray(targets)
    lengths = np.asarray(target_lengths)

    ext = np.zeros((b, s), dtype=np.int64)
    ext[:, 1::2] = targets
    ext_m2 = np.pad(ext[:, :-2], ((0, 0), (2, 0)))
    skip_ok = (np.arange(s)[None, :] >= 2) & (ext != 0) & (ext != ext_m2)
    # states beyond 2*len are invalid; zeroing them in q keeps them exactly 0
    # in the DP so the periodic renorm max is over valid states only
    valid = np.arange(s)[None, :] <= 2 * lengths[:, None]

    raw = np.take_along_axis(pred, ext[:, None, :], axis=2)  # [B, T, S]
    q = np.where(valid[:, None, :], np.exp(raw, dtype=np.float32), 0.0).astype(
        np.float32
    )
    qmax = q.max(axis=2)  # [B, T], > 0 (states 0/1 always valid)
    q /= qmax[:, :, None]
    csum = np.log(qmax.astype(np.float64)).sum(axis=1)  # [B]

    # repeats only matter inside the target length
    rep = targets[:, 1:] == targets[:, :-1]
    inlen = (np.arange(1, l)[None, :] < lengths[:, None])
    has_repeats = bool(np.any(rep & inlen))

    in_maps = []
    if not has_repeats:
        order = np.concatenate([np.arange(1, s, 2), np.arange(0, s, 2)])  # odd|even
        qp = q[:, :, order]
        for k in range(ncores):
            sl = slice(k * bsh, (k + 1) * bsh)
            in_maps.append(
                {
                    "pred": np.ascontiguousarray(pred[sl].reshape(bsh * t, -1)),
                    "qp": np.ascontiguousarray(qp[sl].reshape(bsh, t * s)),
                }
            )
        return "fast", in_maps, csum

    qm = np.where(skip_ok[:, None, :], q, 0.0).astype(np.float32)
    for k in range(ncores):
        sl = slice(k * bsh, (k + 1) * bsh)
        in_maps.append(
            {
                "pred": np.ascontiguousarray(pred[sl].reshape(bsh * t, -1)),
                "q": np.ascontiguousarray(q[sl].reshape(bsh, t * s)),
                "qm": np.ascontiguousarray(qm[sl].reshape(bsh, t * s)),
            }
        )
    return "fallback", in_maps, csum


def host_finish(mode, results, target_lengths, csum, bsh=BSH, t=T, l=L):
    """Combine per-core device outputs into the scalar mean CTC loss."""
    b = len(results) * bsh
    acc = 0.0
    for k, res in enumerate(results):
        a = res["alphaT"].astype(np.float64)
        z = res["zsums"].astype(np.float64)
        r = res["rmaxs"].astype(np.float64)
        logz = np.log(z.T.reshape(-1))  # row-major per-core log Z
        for j in range(bsh):
            bl = int(target_lengths[k * bsh + j])
            lse_sum = logz[j * t : (j + 1) * t].sum()
            logscale = np.log(r[j]).sum() + csum[k * bsh + j]
            if mode == "fast":
                # parity layout: A[2l] = even part col 1+l+bl ; A[2l-1] = col bl
                val = a[j, 1 + l + bl] + a[j, bl]
            else:
                val = a[j, 2 + 2 * bl] + a[j, 2 + 2 * bl - 1]
            with np.errstate(divide="ignore"):
                loss_b = -(np.log(val) + logscale - lse_sum)
            if not np.isfinite(loss_b) or loss_b > 1e29:
                loss_b = 0.0  # zero_infinity
            acc += loss_b / max(bl, 1)
    return np.float32(acc / b)


_NC_CACHE = {}


def _get_nc(mode):
    if mode not in _NC_CACHE:
        _NC_CACHE[mode] = build_fast() if mode == "fast" else build_fallback()
    return _NC_CACHE[mode]


def run_device(mode, in_maps, trace=False, **kwargs):
    nc = _get_nc(mode)
    return bass_utils.run_bass_kernel_spmd(
        nc, in_maps, core_ids=list(range(NCORES)), trace=trace, **kwargs
    )


def kernel(pred, targets, target_lengths):
    pred = np.asarray(pred, dtype=np.float32)
    mode, in_maps, csum = host_prepare(pred, targets, target_lengths)
    res = run_device(mode, in_maps)
    return host_finish(mode, res.results, np.asarray(target_lengths), csum)
